# revision 21
# baseline (speedup 1.0000x reference)
"""RBF kernel layer (retrieval_knn): out = exp(-||x - p||^2) for x [131072, 64]
against 512 prototypes, distributed data-parallel over 8 NeuronCores.
Measured ~83-85us/core at nominal clock (baseline 116-130us).

Math: exp(-dist2) = exp(2*S), S = lhsT.T @ rhs with
  lhsT = [x_t (64); -x_sq/2 hi; -x_sq/2 lo; ones; ones]  (fp16, per point)
  rhs  = [p_t (64); ones; ones; -p_sq/2 hi; -p_sq/2 lo]  (fp16, per proto)
ONE K=68 fp16 matmul per 128-point tile (vs the 2-GEMM bf16 hi/lo
baseline: the PE streams 512 output rows/matmul at ~0.83ns/row
regardless of dtype, so halving the matmul count halves PE time).
fp16 feature rounding gives rel_norm ~2.1e-3 measured (gate 2e-2);
the x_sq/p_sq rows are hi/lo split so their ~32-magnitude values stay
exact, and x_sq/p_sq are computed in f64 on the host.

Output is stored bf16 and upcast on the host, halving the dominant
HBM store traffic (16.8 MB/core). Points are PERMUTED within each
512-row group (tile t, partition p <-> row 4p+t) so each output-store
partition writes 4KB-contiguous bf16 runs.

Steady state is paced equally by the PE (4x427ns matmuls/group, weight
loads fully hidden) and the Scalar engine's 32 exp ACTIVATEs (1964ns
per 4-bank group) - both ~63us, with DMA just below. The schedule
details exist to keep that pipeline airtight: just-in-time input-chunk
DMAs (a matmul's DMA-semaphore threshold covers every DMA issued
before it, so front-loading stalls the start), paired stores (fewer
semaphores -> shorter teardown chain), outp bufs=4 (rides out early
store/load DMA contention), and a split final store (short tail).

Findings from things tried and rejected: f32r matmul = 667ns on HW
(slow weight load); InstActivation off-Scalar is rejected by the BIR
verifier; GpSimd tensor ops are software-emulated (~85us per tile);
a Schraudolph exp-bits tensor_scalar on the DVE works (u16 saturating
convert == bf16 bits, _dve_exp_bits below) but cannot help because the
PE, not Scalar, limits the cadence; fp8 DoubleRow cannot reach 2^-11
precision within the 256-term contraction limit.
"""

import numpy as np

# Problem constants (hardcoded per harness contract; kernel.py is self-contained)
N = 131072
D = 64
M = 512
GAMMA = 1.0
NCORES = 8
NSHARD = N // NCORES  # 16384
P = 128
K1 = D + 4  # contraction: 64 x rows + 2 nxsq rows + 2 ones rows
XCHUNK = 8  # x tiles per input DMA
OCHUNK = 4  # output tiles per ACTIVATE + output DMA (PSUM 4-bank group)
DVE_ACT = False # Schraudolph-on-DVE does not help: the PE paces the pipeline
DVE_PAIRS = 3  # of the 16 o_sb pairs (2 groups each), how many go to DVE
DVE_A = 256.0 * np.log2(np.e)  # 2*log2(e)*128: exp(2S) bits scale
DVE_SIGMA = 0.0434
DVE_B = 128.0 * (127.0 - DVE_SIGMA)

_cache = {}


def _dve_exp_bits(nc, mybir, out_bf16, psum_in):
    """Schraudolph exp on the DVE: bf16 bit pattern of exp(2S) ~
    saturating_round_u16(S * DVE_A + DVE_B). Underflow (y < -127)
    saturates to u16 0 == bf16 +0.0, exactly what exp of a very
    negative argument should round to. One tensor_scalar (mult+add
    fused) per group, ~1-2% relative error on that group's entries."""
    from concourse.alu_op_type import AluOpType

    u16 = mybir.dt.uint16
    return nc.vector.tensor_scalar(
        out_bf16.bitcast(u16), psum_in, DVE_A, DVE_B,
        AluOpType.mult, AluOpType.add,
    )


def _build_bass(nshard=NSHARD):
    import concourse.mybir as mybir
    import concourse.tile as tile
    from concourse import bacc

    f32 = mybir.dt.float32
    f16 = mybir.dt.float16
    bf16 = mybir.dt.bfloat16
    nt = nshard // P
    ngroups = nt // OCHUNK
    assert nt % XCHUNK == 0 and XCHUNK % OCHUNK == 0

    # Whole o_sb pairs go to one engine: mixing engines within a pair
    # serializes Scalar and DVE on the shared output tile. Spread the
    # DVE pairs evenly among the 16.
    npairs = ngroups // 2
    dve_pairs = {
        q for q in range(npairs)
        if ((q + 1) * DVE_PAIRS) // npairs > (q * DVE_PAIRS) // npairs
    } if DVE_ACT else set()
    dve_set = {g for g in range(ngroups) if (g // 2) in dve_pairs}

    nc = bacc.Bacc(None, target_bir_lowering=False)
    # pre-packed on host (already column-permuted): rows 0..63 x features,
    # 64 = -x_sq/2 hi, 65 = lo, 66..67 = ones
    xp_d = nc.dram_tensor("xp", [K1, nshard], f16, kind="ExternalInput")
    # rows 0..63 p features, 64..65 = ones, 66 = -p_sq/2 hi, 67 = lo
    rhs_d = nc.dram_tensor("rhs", [K1, M], f16, kind="ExternalInput")
    out_d = nc.dram_tensor("out", [nshard, M], bf16, kind="ExternalOutput")

    with tile.TileContext(nc) as tc:
        with (
            tc.tile_pool(name="singles", bufs=1) as singles,
            tc.tile_pool(name="outp", bufs=4) as outp,
            tc.tile_pool(name="ps_o", bufs=2, space="PSUM") as ps_o,
        ):
            # rhs via the GpSimd engine's DGE so its descriptor gen runs
            # in parallel with chunk 0's on SP
            rhs_sb = singles.tile([K1, M], f16)
            nc.gpsimd.dma_start(rhs_sb[:], rhs_d[:])

            # all of x stays resident in SBUF (32 KB/partition). Chunk
            # loads are issued just-in-time inside the tile loop: each
            # matmul's semaphore wait covers every DMA issued before it,
            # so front-loading all chunk DMAs would stall the first
            # matmul behind ~17 descriptor generations (~8us). Small
            # first chunks let compute start after ~4 tiles' worth.
            X_all = singles.tile([K1, nt * P], f16)
            bounds = [0, 1, 2, 4, 8, 16]
            while bounds[-1] < nt:
                bounds.append(min(nt, bounds[-1] + 8))
            nchunks = len(bounds) - 1
            next_chunk = 0

            for i in range(nt):
                # chunk 0 issues before tile 0; later chunks stagger one
                # tile apart so the first matmul's DMA-semaphore threshold
                # covers only rhs + chunk 0 (each matmul waits on every
                # DMA issued before it, so front-loading issues stalls
                # the pipeline start).
                while next_chunk < nchunks and max(
                    next_chunk, bounds[next_chunk] - 8
                ) <= i:
                    cs = slice(bounds[next_chunk] * P, bounds[next_chunk + 1] * P)
                    nc.sync.dma_start(X_all[:, cs], xp_d[:, cs])
                    next_chunk += 1
                k = i % OCHUNK
                g = i // OCHUNK
                if i % (2 * OCHUNK) == 0:
                    # one output tile + one store DMA per TWO activation
                    # groups: halves the store count (fewer semaphores ->
                    # shorter end-of-kernel reset chain)
                    o_sb = outp.tile([P, 2, OCHUNK, M], bf16, tag="o")
                if k == 0:
                    psum = ps_o.tile([P, OCHUNK, M], f32, tag="psum")

                nc.tensor.matmul(
                    psum[:, k, :],
                    X_all[:, i * P : (i + 1) * P],
                    rhs_sb[:],
                    start=True,
                    stop=True,
                )

                if k == OCHUNK - 1:
                    h = g % 2
                    if g in dve_set:
                        _dve_exp_bits(nc, mybir, o_sb[:, h], psum[:])
                    else:
                        nc.scalar.activation(
                            o_sb[:, h], psum[:],
                            mybir.ActivationFunctionType.Exp,
                            bias=0.0, scale=2.0,
                        )
                    if g == ngroups - 1:
                        # final pair: store in three pieces (half, then
                        # quarter+quarter) so the tail after the last
                        # ACTIVATE is a single 0.25 MB transfer
                        dest = out_d[
                            (g - 1) * OCHUNK * P : g * OCHUNK * P, :
                        ].rearrange("(p t) m -> p t m", p=P)
                        nc.sync.dma_start(dest, o_sb[:, 0])
                        last = out_d[
                            g * OCHUNK * P : (g + 1) * OCHUNK * P, :
                        ].rearrange("(p t) m -> p t m", p=P)
                        hN = OCHUNK // 2
                        for qq in range(2):
                            nc.sync.dma_start(
                                last[:, qq * hN : (qq + 1) * hN, :],
                                o_sb[:, 1, qq * hN : (qq + 1) * hN],
                            )
                    elif h == 1:
                        # partition p slot t -> row 4p+t within each
                        # 512-row half: with the host-side column
                        # permutation each partition stores two 4KB
                        # contiguous bf16 runs.
                        g0 = g - 1
                        dest = out_d[
                            g0 * OCHUNK * P : (g0 + 2) * OCHUNK * P, :
                        ].rearrange("(h p t) m -> p h t m", h=2, p=P)
                        nc.sync.dma_start(dest, o_sb[:])

    nc.finalize()
    return nc


def _get_nc():
    if "nc" not in _cache:
        _cache["nc"] = _build_bass()
    return _cache["nc"]


def _hilo16(v32):
    h = v32.astype(np.float16)
    l = (v32 - h.astype(np.float32)).astype(np.float16)
    return h, l


def _prep_core_arrays(x, prototypes, nshard):
    """Per-core host arrays: xp [68, nshard] fp16 (column-permuted), rhs
    [68, 512] fp16."""
    x = np.ascontiguousarray(np.asarray(x, dtype=np.float32))
    prototypes = np.ascontiguousarray(np.asarray(prototypes, dtype=np.float32))
    ntotal = x.shape[0]

    nxsq = (-0.5 * (x.astype(np.float64) ** 2).sum(axis=1)).astype(np.float32)
    nxh, nxl = _hilo16(nxsq)
    ones_n = np.ones(ntotal, dtype=np.float16)
    xp_full = np.concatenate(
        [x.T.astype(np.float16), nxh[None], nxl[None], ones_n[None], ones_n[None]],
        axis=0,
    )  # [68, N]

    p_sq = (prototypes.astype(np.float64) ** 2).sum(axis=1)
    nph, npl = _hilo16((-0.5 * p_sq).astype(np.float32))
    ones_m = np.ones((1, M), dtype=np.float16)
    rhs = np.ascontiguousarray(
        np.concatenate(
            [prototypes.T.astype(np.float16), ones_m, ones_m, nph[None], npl[None]],
            axis=0,
        )
    )  # [68, 512]

    # column permutation: within each 512-point block, column t*128+p
    # holds point 4p+t (so tile t partition p <-> output row 4p+t).
    blk = np.arange(OCHUNK * P).reshape(P, OCHUNK).T.ravel()  # [512]
    nblk = nshard // (OCHUNK * P)
    perm = (np.arange(nblk)[:, None] * (OCHUNK * P) + blk[None, :]).ravel()

    ncores = ntotal // nshard
    in_maps = []
    for s in range(ncores):
        cols = s * nshard + perm
        in_maps.append(
            {
                "xp": np.ascontiguousarray(xp_full[:, cols]),
                "rhs": rhs,
            }
        )
    return in_maps


def _prep_inputs(x, prototypes):
    return _prep_core_arrays(x, prototypes, NSHARD)


def _run(inputs, trace=False):
    from concourse.bass_utils import run_bass_kernel_spmd

    in_maps = _prep_inputs(inputs["x"], inputs["prototypes"])
    nc = _get_nc()
    res = run_bass_kernel_spmd(
        nc, in_maps, core_ids=list(range(NCORES)), trace=trace
    )
    out = np.concatenate(
        [np.asarray(r["out"]).astype(np.float32) for r in res.results], axis=0
    )
    return out, res


def kernel(**inputs) -> np.ndarray:
    out, _ = _run(inputs, trace=False)
    return out
